# revision 56
# baseline (speedup 1.0000x reference)
"""MLA attention TRN2 kernel: 4-way data-parallel (sequences) x 2-way
tensor-parallel (heads). Each core: 1 sequence (1024 tokens), 8 heads.

v2: all matmul operands bf16 (fp32 PSUM accumulation), host-side
pre-tiled weights (contiguous per-partition DMA lines, ~6x fewer
descriptors), token-major x staging so q_a starts ~7us in, and
precomputed per-head rope-q / k_nope so the attention chunk loops
stream back-to-back on the PE (keeps the tensor engine at max p-state).

Layout convention: features on partitions, tokens on free axis; scores
computed transposed [k, q] so softmax sums use PE ones-matmuls and no
transposes are needed anywhere. Softmax denominator folded into avT via
per-(head, qblock-pair) broadcast multiply.
"""
import sys
sys.path.insert(0, '/opt/trn_rl_repo')

import math
import numpy as np

import concourse.bass as bass
import concourse.tile as tile
from concourse.tile_rust import add_dep_helper
from concourse import bacc, mybir

BF = mybir.dt.bfloat16
F32 = mybir.dt.float32
AF = mybir.ActivationFunctionType

H = 16
NH = 8            # heads per core
NOPE = 128
ROPE = 64
VD = 128
HID = 2048
QLR = 1536
KVLR = 512
B = 4
S = 1024
EPS = 1e-6
SCALE = 1.0 / math.sqrt(NOPE + ROPE)
NEG = -1.0e30

HID_T = HID // 128    # 16
QLR_T = QLR // 128    # 12
KVLR_T = KVLR // 128  # 4
QF = NH * (NOPE + ROPE)   # 1536 q features per core
QF_T = QF // 128          # 12 (chunks 0..7 nope, 8..11 rope)
NTC = S // 128            # 8 token chunks
QB = 256                  # query block
NQB = S // QB             # 4


def build_nc():
    nc = bacc.Bacc("TRN2", target_bir_lowering=False, debug=False, num_devices=8)

    xW = nc.dram_tensor("xW", [128, 2, HID_T, 512], BF, kind="ExternalInput")
    wqa_t = nc.dram_tensor("wqa_t", [QLR_T, 128, HID_T, 128], BF,
                           kind="ExternalInput")
    wkva_t = nc.dram_tensor("wkva_t", [4, 128, HID_T, 128], BF,
                            kind="ExternalInput")
    wkva_r = nc.dram_tensor("wkva_r", [128, HID_T, ROPE], BF,
                            kind="ExternalInput")
    wqb_t = nc.dram_tensor("wqb_t", [QF_T, 128, QLR_T, 128], BF,
                           kind="ExternalInput")
    wkb_t = nc.dram_tensor("wkb_t", [KVLR_T, 128, NH * NOPE], BF,
                           kind="ExternalInput")
    wvb_t = nc.dram_tensor("wvb_t", [KVLR_T, 128, NH * VD], BF,
                           kind="ExternalInput")
    wo_t = nc.dram_tensor("wo_t", [4, 128, NH, 512], BF, kind="ExternalInput")
    cosT = nc.dram_tensor("cosT", [ROPE // 2, S], BF, kind="ExternalInput")
    sinT = nc.dram_tensor("sinT", [ROPE // 2, S], BF, kind="ExternalInput")
    masks = nc.dram_tensor("masks", [2, 128, QB], F32, kind="ExternalInput")
    ones_col = nc.dram_tensor("ones_col", [128, 1], mybir.dt.float32r,
                              kind="ExternalInput")
    ones_row = nc.dram_tensor("ones_row", [1, 128], BF, kind="ExternalInput")
    ones_sq = nc.dram_tensor("ones_sq", [128, 128], BF, kind="ExternalInput")
    out = nc.dram_tensor("out", [S, HID], F32, kind="ExternalOutput")

    with tile.TileContext(nc) as tc:
        build_body(tc, xW=xW, wqa_t=wqa_t, wkva_t=wkva_t, wkva_r=wkva_r,
                   wqb_t=wqb_t, wkb_t=wkb_t, wvb_t=wvb_t, wo_t=wo_t,
                   cosT=cosT, sinT=sinT, masks=masks, ones_col=ones_col,
                   ones_row=ones_row, ones_sq=ones_sq, out=out)
    nc.compile()
    return nc


def build_body(tc, *, xW, wqa_t, wkva_t, wkva_r, wqb_t, wkb_t, wvb_t, wo_t,
               cosT, sinT, masks, ones_col, ones_row, ones_sq, out):
    from contextlib import ExitStack
    nc = tc.nc

    with (
        tc.tile_pool(name="const", bufs=1) as pconst,
        tc.tile_pool(name="ckv", bufs=1) as pckv,
    ):
        # const tiles created here; their DMAs are issued AFTER the
        # startup-critical x/wqa stream (none are needed before q_b).
        ones_c = pconst.tile([128, 1], mybir.dt.float32r, name="ones_c")
        ones_r = pconst.tile([1, 128], BF, name="ones_r")
        ones_s = pconst.tile([128, 128], BF, name="ones_s")
        mask_sb = [pconst.tile([128, QB], F32, name=f"mask{i}")
                   for i in range(2)]
        cos_sb = pconst.tile([ROPE // 2, S], BF, name="cos_sb")
        sin_sb = pconst.tile([ROPE // 2, S], BF, name="sin_sb")

        def emit_const_dmas():
            nc.sync.dma_start(ones_c[:], ones_col[:])
            nc.sync.dma_start(ones_r[:], ones_row[:])
            nc.sync.dma_start(ones_s[:], ones_sq[:])
            for i in range(2):
                nc.sync.dma_start(mask_sb[i][:], masks[i])
            nc.sync.dma_start(cos_sb[:], cosT[:])
            nc.sync.dma_start(sin_sb[:], sinT[:])
        # rope contract zero-padded to 128 partitions: 64-contract matmuls
        # measure ~280ns vs 216ns for 128-contract at 512 moving rows.
        kpe = pconst.tile([128, S], BF, name="kpe")
        nc.vector.memset(kpe[ROPE:128, :], 0.0)
        eps_t = pconst.tile([1, 1], F32, name="eps_t")
        nc.vector.memset(eps_t[:], EPS)

        ckv = [pckv.tile([128, S], BF, name=f"ckv{i}") for i in range(KVLR_T)]

        # ============ Phases: q_a -> kv_a (norms overlap) -> q_b ============
        es_u = ExitStack()
        pu = es_u.enter_context(tc.tile_pool(name="u", bufs=1))
        es_qbc = ExitStack()
        pqbc = es_qbc.enter_context(tc.tile_pool(name="qbc", bufs=1))
        qbc = [pqbc.tile([128, 512], BF, name=f"qbcn{i}") for i in range(2)]
        # cross-phase rmsnorm state: f32 square-sum accumulators (filled on
        # DVE during q_a/kv_a; reduced + broadcast inside the q_b stream) and
        # the raw k_pe rows.
        es_nrm = ExitStack()
        pnrm2 = es_nrm.enter_context(tc.tile_pool(name="nrm2", bufs=1))
        z2qa = [pnrm2.tile([128, 512], mybir.dt.float32r, name=f"z2qa{h}")
                for h in range(2)]
        z2ka = [pnrm2.tile([128, 512], mybir.dt.float32r, name=f"z2ka{h}")
                for h in range(2)]
        kv4 = pnrm2.tile([ROPE, S], BF, name="kv4")
        es_x = ExitStack()
        px = es_x.enter_context(tc.tile_pool(name="xsb", bufs=1))

        # token-major x staging: wave w = tokens [w*512, (w+1)*512) for all
        # of HID, so q_a half 0 can start as soon as wave 0 lands.
        xsb = px.tile([128, 2, HID_T, 512], BF, name="xsb")

        es_wqa = ExitStack()
        pwqa = es_wqa.enter_context(tc.tile_pool(name="wqa", bufs=1))
        wqa_s = [pwqa.tile([128, HID_T, 128], BF, name=f"wqa_s{m}")
                 for m in range(QLR_T)]
        # startup-critical DMA order: interleave strip0/wave0 in ko-quarters
        # (the first q_a group consumes ko 0..15 serially), then strip 1,
        # strips 2-11, consts, wave 1.
        for sub in range(4):
            ksl = slice(4 * sub, 4 * sub + 4)
            nc.sync.dma_start(wqa_s[0][:, ksl, :], wqa_t[0][:, ksl, :])
            nc.sync.dma_start(xsb[:, 0, ksl, :], xW[:, 0, ksl, :])
        nc.sync.dma_start(wqa_s[1][:], wqa_t[1])
        for m in range(2, QLR_T):
            nc.sync.dma_start(wqa_s[m][:], wqa_t[m])
        emit_const_dmas()
        nc.sync.dma_start(xsb[:, 1], xW[:, 1])

        u = [pu.tile([128, S], BF, name=f"u{i}") for i in range(QLR_T)]

        with (
            tc.tile_pool(name="wstripA", bufs=3) as pwA,
            tc.tile_pool(name="sqA", bufs=4) as psq,
            tc.tile_pool(name="ppmain", bufs=4, space="PSUM") as ppm,
            tc.tile_pool(name="ppwarm", bufs=1, space="PSUM") as ppw,
        ):
            # ---- PE p-state warm-up: run dummy matmuls on a memset tile
            # while the startup DMAs land, so the tensor engine is at max
            # clock (needs ~3us continuous busy) when q_a starts. ----
            dz = psq.tile([128, 512], BF, name="dz", tag="warmz", bufs=1)
            nc.vector.memset(dz[:], 0.0)
            ps_w = ppw.tile([128, 512], F32, name="ps_warm", tag="warm")
            for i in range(6):
                nc.tensor.matmul(ps_w[:], dz[:, 0:128], dz[:],
                                 skip_group_check=True)

            # ---- q_a: u = wqa.T @ x -> [1536, S], half-major over tokens ----
            # square-sums accumulate on DVE (f32 SBUF acc); the partition
            # reduce + rsqrt + broadcast run later, inside the q_b stream.
            with nc.named_scope("q_a"):
                for half in range(2):
                    sl = slice(half * 512, (half + 1) * 512)
                    for m in range(QLR_T):
                        ps = ppm.tile([128, 512], F32, name=f"psu{m}_{half}",
                                      tag="main")
                        for ko in range(HID_T):
                            nc.tensor.matmul(
                                ps[:], wqa_s[m][:, ko, :], xsb[:, half, ko, :],
                                start=(ko == 0), stop=(ko == HID_T - 1))
                        cp = nc.scalar.copy(u[m][:, sl], ps[:])
                        if m == 0 and half == 0:
                            gate_qa = cp
                        sq = psq.tile([128, 512], BF, name=f"squ{m}_{half}",
                                      tag="sq")
                        nc.scalar.activation(sq[:], ps[:], AF.Square)
                        if m == 0:
                            nc.vector.tensor_copy(z2qa[half][:], sq[:])
                        else:
                            nc.vector.tensor_add(z2qa[half][:], z2qa[half][:],
                                                 sq[:])

            # ---- kv_a: kvT = wkva.T @ x -> [576, S] ----
            with nc.named_scope("kv_a"):
                for m in range(5):
                    mw = 128 if m < 4 else ROPE
                    ws = pwA.tile([128, HID_T, mw], BF, name=f"wkva_s{m}",
                                  tag="wstrip")
                    nc.sync.dma_start(ws[:], wkva_t[m] if m < 4 else wkva_r[:])
                    for half in range(2):
                        sl = slice(half * 512, (half + 1) * 512)
                        ps = ppm.tile([128, 512], F32, name=f"pskv{m}_{half}",
                                      tag="main")
                        for ko in range(HID_T):
                            nc.tensor.matmul(
                                ps[:mw, :], ws[:, ko, :], xsb[:, half, ko, :],
                                start=(ko == 0), stop=(ko == HID_T - 1))
                        if m < 4:
                            nc.scalar.copy(ckv[m][:, sl], ps[:])
                            sq = psq.tile([128, 512], BF, name=f"sqk{m}_{half}",
                                          tag="sq")
                            sq_act = nc.scalar.activation(sq[:], ps[:],
                                                          AF.Square)
                            if m == 3:
                                last_square = sq_act
                            if m == 0:
                                nc.vector.tensor_copy(z2ka[half][:], sq[:])
                            else:
                                nc.vector.tensor_add(z2ka[half][:],
                                                     z2ka[half][:], sq[:])
                        else:
                            nc.scalar.copy(kv4[:, sl], ps[:mw, :])
                # pre-warm the rsqrt activation table after the last Square
                # so norm_stage's rsqrt does not pay the ~1.3us table load
                rsw = psq.tile([1, 1], BF, name="rsqwarm", tag="rsw", bufs=1)
                warm = nc.scalar.activation(rsw[:], eps_t[:],
                                            AF.Abs_reciprocal_sqrt)
                add_dep_helper(warm.ins, last_square.ins, sync=True,
                               reason="rsqrt table warm-up after last Square")

        # ---- q_b: qT = wqb.T @ u -> [1536, S] (x, wqa freed). The rmsnorm
        # reduce/rsqrt/broadcast work is staggered into the first three q_b
        # groups so the PE never waits on the scalar-engine norm chain. ----
        es_wqa.close()
        es_x.close()
        es_qT = ExitStack()
        pqT = es_qT.enter_context(tc.tile_pool(name="qT", bufs=1, side="right"))
        qT = [pqT.tile([128, S], BF, name=f"qTt{i}") for i in range(QF_T)]
        with (
            tc.tile_pool(name="wstripB", bufs=3) as pwB,
            tc.tile_pool(name="normB", bufs=2) as pnorm,
            tc.tile_pool(name="ppmainB", bufs=4, space="PSUM") as ppmB,
            tc.tile_pool(name="ppz2", bufs=2, space="PSUM") as ppz2,
            tc.tile_pool(name="ppbc", bufs=2, space="PSUM") as ppbc,
        ):
            z2q = [ppz2.tile([1, 512], F32, name=f"z2q{h}", tag="z2")
                   for h in range(2)]
            z2k = [ppz2.tile([1, 512], F32, name=f"z2k{h}", tag="z2")
                   for h in range(2)]
            rrq = [pnorm.tile([1, 512], BF, name=f"qrr{h}", tag="rr")
                   for h in range(2)]
            rrk = [pnorm.tile([1, 512], BF, name=f"krr{h}", tag="rrk")
                   for h in range(2)]

            def emit_rsqrt(rr, z2, n, nm):
                nc.scalar.activation(rr[:], z2[:], AF.Abs_reciprocal_sqrt,
                                     scale=1.0 / n, bias=eps_t[:])

            def norm_stage(m):
                if m == 0:
                    # partition-reduce q square-sums; rsqrt chain
                    for half in range(2):
                        nc.tensor.matmul(z2q[half][:], ones_c[:],
                                         z2qa[half][:], skip_group_check=True)
                        emit_rsqrt(rrq[half], z2q[half], QLR, f'q{half}')
                elif m == 5:
                    for half in range(2):
                        pb = ppbc.tile([128, 512], F32, name=f"qpb{half}",
                                       tag="bc")
                        nc.tensor.matmul(pb[:], ones_r[:], rrq[half][:],
                                         skip_group_check=True)
                        nc.scalar.copy(qbc[half][:], pb[:])
                        nc.tensor.matmul(z2k[half][:], ones_c[:],
                                         z2ka[half][:], skip_group_check=True)
                        emit_rsqrt(rrk[half], z2k[half], KVLR, f'k{half}')
                elif m == 8:
                    with nc.named_scope("ckv_norm"):
                        for half in range(2):
                            sl = slice(half * 512, (half + 1) * 512)
                            pb = ppbc.tile([128, 512], F32, name=f"kpb{half}",
                                           tag="bc")
                            nc.tensor.matmul(pb[:], ones_r[:], rrk[half][:],
                                             skip_group_check=True)
                            bc = pnorm.tile([128, 512], BF, name=f"kbc{half}",
                                            tag="kbc")
                            nc.scalar.copy(bc[:], pb[:])
                            for mm in range(KVLR_T):
                                nc.vector.tensor_mul(ckv[mm][:, sl],
                                                     ckv[mm][:, sl], bc[:])
                        # RoPE for k_pe; DVE needs equal base partitions
                        e = kv4[0:32, :]
                        ko0 = pnorm.tile([32, S], BF, name="ko0", tag="r32a",
                                         bufs=1)
                        nc.gpsimd.dma_start(ko0[:], kv4[32:64, :])
                        kt1 = pnorm.tile([32, S], BF, name="kt1", tag="r32b",
                                         bufs=1)
                        nc.vector.tensor_mul(kt1[:], e, cos_sb[:])
                        kt2 = pnorm.tile([32, S], BF, name="kt2", tag="r32c",
                                         bufs=1)
                        nc.vector.tensor_mul(kt2[:], ko0[:], sin_sb[:])
                        nc.vector.tensor_sub(kpe[0:32, :], kt1[:], kt2[:])
                        kt3 = pnorm.tile([32, S], BF, name="kt3", tag="r32b",
                                         bufs=1)
                        nc.vector.tensor_mul(kt3[:], e, sin_sb[:])
                        kt4 = pnorm.tile([32, S], BF, name="kt4", tag="r32c",
                                         bufs=1)
                        nc.vector.tensor_mul(kt4[:], ko0[:], cos_sb[:])
                        ktO = pnorm.tile([32, S], BF, name="ktO", tag="r32a",
                                         bufs=1)
                        nc.vector.tensor_add(ktO[:], kt3[:], kt4[:])
                        nc.gpsimd.dma_start(kpe[32:64, :], ktO[:])

            with nc.named_scope("q_b"):
                # qbc is produced at norm_stage(1); the norm muls for the
                # first two strips must be EMITTED after that write (an
                # earlier-emitted read would be a WAR on stale qbc).
                mul_pend = []
                for m in range(QF_T):
                    ws = pwB.tile([128, QLR_T, 128], BF, name=f"wqb_s{m}",
                                  tag="wstripB")
                    dma = nc.sync.dma_start(ws[:], wqb_t[m])
                    add_dep_helper(dma.ins, gate_qa.ins, sync=True,
                                   reason="stage wqb DMA after q_a starts")
                    norm_stage(m)
                    if m == 6:
                        for fn in mul_pend:
                            fn()
                        mul_pend = []
                    if m == 7:
                        # pre-warm the EXP activation table off the critical
                        # path (the load costs ~1.3us on the scalar engine)
                        warm = pnorm.tile([1, 512], BF, name="expwarm",
                                          tag="rt", bufs=1)
                        nc.scalar.activation(warm[:], z2q[0][:], AF.Exp,
                                             scale=SCALE)
                    for half in range(2):
                        sl = slice(half * 512, (half + 1) * 512)
                        ps = ppmB.tile([128, 512], F32, name=f"psq{m}_{half}",
                                       tag="mainB")
                        for ko in range(QLR_T):
                            nc.tensor.matmul(
                                ps[:], ws[:, ko, :], u[ko][:, sl],
                                start=(ko == 0), stop=(ko == QLR_T - 1))
                        cp = nc.scalar.copy(qT[m][:, sl], ps[:])
                        if m < 6:
                            mul_pend.append(
                                lambda m=m, sl=sl, half=half:
                                nc.vector.tensor_mul(qT[m][:, sl],
                                                     qT[m][:, sl],
                                                     qbc[half][:]))
                        else:
                            nc.vector.tensor_mul(qT[m][:, sl], qT[m][:, sl],
                                                 qbc[half][:])
                        if m == 0 and half == 0:
                            gate_qb = cp
        es_nrm.close()
        es_qbc.close()
        es_u.close()

        # pools in reverse-close order: avT (o_proj) under qpeA/knA
        # (attention) under qrope (released right after q_rope consumers).
        es_avT = ExitStack()
        pavT = es_avT.enter_context(tc.tile_pool(name="avT", bufs=1))
        avT = [pavT.tile([128, S], BF, name=f"avT{i}") for i in range(NH)]
        es_qpe = ExitStack()
        pqpeA = es_qpe.enter_context(tc.tile_pool(name="qpeA", bufs=1))
        qpe_all = pqpeA.tile([128, NH, S], BF, name="qpe_all")
        nc.vector.memset(qpe_all[ROPE:128], 0.0)
        es_knA = ExitStack()
        pknA = es_knA.enter_context(tc.tile_pool(name="knA", bufs=1))
        kn_all = pknA.tile([128, NH, S], BF, name="kn_all")
        es_qr = ExitStack()
        pqr = es_qr.enter_context(tc.tile_pool(name="qrope", bufs=1))

        # ---- RoPE for all heads' q_pe upfront (overlaps v_b/kn below) ----
        with nc.named_scope("q_rope"):
            # 4 heads stacked on partitions (32 rows each), 2 head-groups on
            # the free axis: qe4[32*i:32*i+32, jj, :] = evens of head jj*4+i.
            qe4 = pqr.tile([128, 2, S], BF, name="qe4")
            qo4 = pqr.tile([128, 2, S], BF, name="qo4")
            c4 = pqr.tile([128, 2, S], BF, name="c4")
            s4 = pqr.tile([128, 2, S], BF, name="s4")
            for i in range(4):
                for jj in range(2):
                    nc.gpsimd.dma_start(c4[32 * i:32 * i + 32, jj, :],
                                        cos_sb[:])
                    nc.gpsimd.dma_start(s4[32 * i:32 * i + 32, jj, :],
                                        sin_sb[:])
            for h in range(NH):
                j, b2 = h // 2, h % 2
                base = b2 * 64
                i, jj = h % 4, h // 4
                nc.gpsimd.dma_start(qe4[32 * i:32 * i + 32, jj, :],
                                    qT[NH + j][base:base + 32, :])
                nc.gpsimd.dma_start(qo4[32 * i:32 * i + 32, jj, :],
                                    qT[NH + j][base + 32:base + 64, :])
            ta = pqr.tile([128, 2, S], BF, name="qta", tag="ta", bufs=1)
            nc.vector.tensor_mul(ta[:], qe4[:], c4[:])
            tb = pqr.tile([128, 2, S], BF, name="qtb", tag="tb", bufs=1)
            nc.vector.tensor_mul(tb[:], qo4[:], s4[:])
            qpeE = pqr.tile([128, 2, S], BF, name="qpeE")
            nc.vector.tensor_sub(qpeE[:], ta[:], tb[:])
            ta2 = pqr.tile([128, 2, S], BF, name="qta2", tag="ta", bufs=1)
            nc.vector.tensor_mul(ta2[:], qe4[:], s4[:])
            tb2 = pqr.tile([128, 2, S], BF, name="qtb2", tag="tb", bufs=1)
            nc.vector.tensor_mul(tb2[:], qo4[:], c4[:])
            qpeO = pqr.tile([128, 2, S], BF, name="qpeO")
            nc.vector.tensor_add(qpeO[:], ta2[:], tb2[:])
            for h in range(NH):
                i, jj = h % 4, h // 4
                nc.gpsimd.dma_start(qpe_all[0:32, h, :],
                                    qpeE[32 * i:32 * i + 32, jj, :])
                nc.gpsimd.dma_start(qpe_all[32:64, h, :],
                                    qpeO[32 * i:32 * i + 32, jj, :])

        # ======================= attention =======================
        with tc.tile_pool(name="wkb", bufs=1) as pwkb:
            wkb = [pwkb.tile([128, NH * NOPE], BF, name=f"wkb{i}")
                   for i in range(KVLR_T)]
            for ko in range(KVLR_T):
                dma = nc.sync.dma_start(wkb[ko][:], wkb_t[ko])
                add_dep_helper(dma.ins, gate_qb.ins, sync=True,
                               reason="stage wkb DMA after q_b starts")

            with tc.tile_pool(name="v", bufs=1) as pv:
                v_sb = [[pv.tile([128, 512], BF, name=f"v{g}_{t}")
                         for t in range(NTC)] for g in range(2)]
                with (
                    tc.tile_pool(name="wvb", bufs=1) as pwvb,
                    tc.tile_pool(name="ppv", bufs=4, space="PSUM") as ppv,
                    tc.tile_pool(name="ppknb", bufs=2, space="PSUM") as ppknb,
                ):
                    wvb = [pwvb.tile([128, NH * VD], BF, name=f"wvb{i}")
                           for i in range(KVLR_T)]
                    for ko in range(KVLR_T):
                        dma = nc.sync.dma_start(wvb[ko][:], wvb_t[ko])
                        add_dep_helper(dma.ins, gate_qb.ins, sync=True,
                                       reason="stage wvb DMA after q_b starts")
                    with nc.named_scope("v_b"):
                        for g in range(2):
                            for t in range(NTC):
                                ps = ppv.tile([128, 512], F32,
                                              name=f"psv{g}_{t}", tag="v")
                                for kk in range(KVLR_T):
                                    nc.tensor.matmul(
                                        ps[:],
                                        ckv[kk][:, t * 128:(t + 1) * 128],
                                        wvb[kk][:, g * 512:(g + 1) * 512],
                                        start=(kk == 0),
                                        stop=(kk == KVLR_T - 1))
                                nc.scalar.copy(v_sb[g][t][:], ps[:])
                    with nc.named_scope("k_b"):
                        for h in range(NH):
                            for half in range(2):
                                sl = slice(half * 512, (half + 1) * 512)
                                ps = ppknb.tile([128, 512], F32,
                                                name=f"pskn{h}_{half}",
                                                tag="kn")
                                for kk in range(KVLR_T):
                                    nc.tensor.matmul(
                                        ps[:],
                                        wkb[kk][:, h * 128:(h + 1) * 128],
                                        ckv[kk][:, sl],
                                        start=(kk == 0),
                                        stop=(kk == KVLR_T - 1))
                                nc.vector.tensor_copy(kn_all[:, h, sl], ps[:])

                with (
                    tc.tile_pool(name="pbuf", bufs=4) as ppbuf,
                    tc.tile_pool(name="zbuf", bufs=2) as pzbuf,
                    tc.tile_pool(name="ppsc", bufs=4, space="PSUM") as ppsc,
                    tc.tile_pool(name="ppav", bufs=2, space="PSUM") as ppav,
                    tc.tile_pool(name="ppz", bufs=2, space="PSUM") as ppz,
                ):
                    # the last chunk's av/z matmuls and pair-1 divide of each
                    # head carry into the next head's emission so the PE does
                    # not wait on the final exp at head boundaries
                    carry = []
                    for h in range(NH):
                        with nc.named_scope(f"attn_h{h}"):
                            amul, carry = attention_head(
                                tc, h, qT=qT, kn_all=kn_all, kpe=kpe,
                                qpe_all=qpe_all, v_sb=v_sb, avT=avT,
                                mask_sb=mask_sb, ones_s=ones_s,
                                ppbuf=ppbuf, pzbuf=pzbuf, ppsc=ppsc,
                                ppav=ppav, ppz=ppz, carry=carry)
                            if h == 0:
                                gate_attn = amul
                    for fn in carry:
                        fn()

                    # ====== o_proj (inside the attention pool scope so its
                    # PSUM tiles rotate through the score banks — freed by
                    # the last head's exps, not its divide chain — and no
                    # pool-transition barrier sits on the handoff) ======
                    with (
                        tc.tile_pool(name="wo", bufs=4) as pwo,
                        tc.tile_pool(name="osb", bufs=3) as posb,
                    ):
                        with nc.named_scope("o_proj"):
                            for hc in range(4):
                                ws = pwo.tile([128, NH, 512], BF,
                                              name=f"wo_s{hc}", tag="wo")
                                dma = nc.sync.dma_start(ws[:], wo_t[hc])
                                add_dep_helper(
                                    dma.ins, gate_attn.ins, sync=True,
                                    reason="stage wo DMA after attention "
                                           "starts")
                                for t in range(NTC):
                                    ps = ppsc.tile([128, 512], F32,
                                                   name=f"pso{hc}_{t}",
                                                   tag="sc")
                                    for kk in range(NH):
                                        nc.tensor.matmul(
                                            ps[:],
                                            avT[kk][:, t * 128:(t + 1) * 128],
                                            ws[:, kk, :],
                                            start=(kk == 0),
                                            stop=(kk == NH - 1))
                                    ot = posb.tile([128, 512], F32,
                                                   name=f"ot{hc}_{t}",
                                                   tag="ot")
                                    nc.scalar.copy(ot[:], ps[:])
                                    nc.gpsimd.dma_start(
                                        out[t * 128:(t + 1) * 128,
                                            hc * 512:(hc + 1) * 512], ot[:])
        es_qr.close()
        es_knA.close()
        es_qpe.close()
        es_qT.close()
        es_avT.close()


def attention_head(tc, h, *, qT, kn_all, kpe, qpe_all, v_sb, avT, mask_sb,
                   ones_s, ppbuf, pzbuf, ppsc, ppav, ppz, carry):
    nc = tc.nc

    # Two query-block pairs; pair p covers qblocks (2p, 2p+1) at columns
    # p*512..(p+1)*512. Chunk loop shared so the kn/kpe stationaries are
    # reused back-to-back across pairs.
    st = []
    for p in range(2):
        q0 = 2 * p
        nw = 2 * q0 + 2
        st.append(dict(
            q0=q0, nw=nw, nk=nw + 2,
            psl=slice(p * 512, (p + 1) * 512),
            nsl=slice(p * 512 + 256, p * 512 + 512),
            ps_av=ppav.tile([128, 512], F32, name=f"psav{h}_{p}", tag="av"),
            ps_z=ppz.tile([128, 512], F32, name=f"psz{h}_{p}", tag="z"),
        ))
    pend = list(carry)

    def finish_chunk(p, kc, ps_s):
        s = st[p]
        wide = kc < s["nw"]
        cn = 512 if wide else 256
        d = kc - 2 * s["q0"] if wide else kc - 2 * (s["q0"] + 1)
        if d >= 0:
            nc.vector.tensor_add(ps_s[:, 0:256], ps_s[:, 0:256],
                                 mask_sb[d][:])
        p_sb = ppbuf.tile([128, 512], BF, name=f"p{h}_{p}_{kc}", tag="p")
        nc.scalar.activation(p_sb[:, :cn], ps_s[:, :cn], AF.Exp, scale=SCALE)

        def avz():
            vt = v_sb[h // 4][kc][:, (h % 4) * 128:(h % 4 + 1) * 128]
            osl = slice(0, 512) if wide else slice(256, 512)
            nc.tensor.matmul(s["ps_av"][:, osl], vt, p_sb[:, :cn],
                             start=(kc == 0), stop=(kc == s["nk"] - 1),
                             skip_group_check=True)
            nc.tensor.matmul(s["ps_z"][:, osl], ones_s[:], p_sb[:, :cn],
                             start=(kc == 0), stop=(kc == s["nk"] - 1),
                             skip_group_check=True)
        return avz

    def divide(p):
        s = st[p]
        zr = pzbuf.tile([128, 512], F32, name=f"zr{h}_{p}", tag="zr")
        nc.vector.reciprocal_approx_fast(zr[:], s["ps_z"][:])
        return nc.vector.tensor_mul(avT[h][:, s["psl"]], s["ps_av"][:],
                                    zr[:])

    for kc in range(st[1]["nk"]):
        ksl = slice(kc * 128, (kc + 1) * 128)
        active = [p for p in range(2) if kc < st[p]["nk"]]
        tiles = {}
        # nope score matmuls first (shared kn stationary), then rope
        # (shared kpe stationary).
        for p in active:
            s = st[p]
            wide = kc < s["nw"]
            cn = 512 if wide else 256
            csl = s["psl"] if wide else s["nsl"]
            ps_s = ppsc.tile([128, 512], F32, name=f"pss{h}_{p}_{kc}",
                             tag="sc")
            tiles[p] = ps_s
            nc.tensor.matmul(ps_s[:, :cn], kn_all[:, h, ksl],
                             qT[h][:, csl], start=True, stop=False,
                             skip_group_check=True)
        for p in active:
            s = st[p]
            wide = kc < s["nw"]
            cn = 512 if wide else 256
            csl = s["psl"] if wide else s["nsl"]
            nc.tensor.matmul(tiles[p][:, :cn], kpe[:, ksl],
                             qpe_all[:, h, csl], start=False, stop=True,
                             skip_group_check=True)
        cur = [finish_chunk(p, kc, tiles[p]) for p in active]
        for fn in pend:
            fn()
        pend = cur
        if kc == st[0]["nk"]:
            # pair 0's accumulation completed in the flush above; divide it
            # now so its PSUM banks free early (shortens the o_proj handoff)
            mul0 = divide(0)
    leftover = list(pend) + [lambda: divide(1)]
    return mul0, leftover


# ---------------------------------------------------------------------------
# Host-side prep
# ---------------------------------------------------------------------------

def _strip_tile(wT, sw):
    """[K, W] -> [W//sw, 128, K//128, sw]: per-strip contiguous SBUF images."""
    K, W = wT.shape
    ko_n = K // 128
    n = W // sw
    return np.ascontiguousarray(
        wT.reshape(ko_n, 128, n, sw).transpose(2, 1, 0, 3))


def prepare_inputs(inputs: dict) -> list[dict]:
    """Full problem inputs -> list of 8 per-core input maps."""
    import ml_dtypes
    bf16 = ml_dtypes.bfloat16

    x = np.asarray(inputs["x"], np.float32)
    wq_a = np.asarray(inputs["wq_a"], np.float32)
    w_qa_ln = np.asarray(inputs["w_qa_ln"], np.float32)
    wq_b = np.asarray(inputs["wq_b"], np.float32)
    wkv_a = np.asarray(inputs["wkv_a"], np.float32)
    w_kva_ln = np.asarray(inputs["w_kva_ln"], np.float32)
    wk_b = np.asarray(inputs["wk_b"], np.float32)
    wv_b = np.asarray(inputs["wv_b"], np.float32)
    wo = np.asarray(inputs["wo"], np.float32)
    rotary_sin = np.asarray(inputs["rotary_sin"], np.float32)
    rotary_cos = np.asarray(inputs["rotary_cos"], np.float32)

    wqaT = np.ascontiguousarray(wq_a.T)                      # [HID, QLR]
    kv_perm = (list(range(KVLR))
               + [KVLR + 2 * i for i in range(ROPE // 2)]
               + [KVLR + 2 * i + 1 for i in range(ROPE // 2)])
    wkvaT = np.ascontiguousarray(wkv_a[kv_perm, :].T)        # [HID, 576]

    wq_b_eff = wq_b * w_qa_ln[None, :]
    wk_b_eff = wk_b * w_kva_ln[None, :]
    wv_b_eff = wv_b * w_kva_ln[None, :]

    wqa_t = _strip_tile(wqaT, 128).astype(bf16)              # [12,128,16,128]
    wkva_full = _strip_tile(wkvaT[:, :KVLR], 128).astype(bf16)  # [4,128,16,128]
    wkva_r = _strip_tile(wkvaT[:, KVLR:], ROPE)[0].astype(bf16)  # [128,16,64]

    per_group = []
    for g in range(2):
        heads = range(g * NH, (g + 1) * NH)
        qperm = [h * (NOPE + ROPE) + d for h in heads for d in range(NOPE)]
        for h in heads:
            qperm += [h * (NOPE + ROPE) + NOPE + 2 * i for i in range(ROPE // 2)]
            qperm += [h * (NOPE + ROPE) + NOPE + 2 * i + 1
                      for i in range(ROPE // 2)]
        wqbT = np.ascontiguousarray(wq_b_eff[qperm, :].T)    # [QLR, 1536]
        wqb_t = _strip_tile(wqbT, 128).astype(bf16)          # [12,128,12,128]
        cols = [h * NOPE + d for h in heads for d in range(NOPE)]
        wkbT = wk_b_eff[cols, :].T                           # [KVLR, 1024]
        wkb_t = np.ascontiguousarray(
            wkbT.reshape(KVLR_T, 128, NH * NOPE)).astype(bf16)
        wvbT = wv_b_eff[cols, :].T
        wvb_t = np.ascontiguousarray(
            wvbT.reshape(KVLR_T, 128, NH * VD)).astype(bf16)
        woT = np.ascontiguousarray(wo[:, cols].T)            # [1024, HID]
        wo_t = _strip_tile(woT, 512).astype(bf16)            # [4,128,8,512]
        wo_t = np.ascontiguousarray(wo_t)
        per_group.append((wqb_t, wkb_t, wvb_t, wo_t))

    cosT = np.ascontiguousarray(rotary_cos.T).astype(bf16)   # [32, S]
    sinT = np.ascontiguousarray(rotary_sin.T).astype(bf16)
    kq = np.arange(128)[:, None]
    qq = np.arange(QB)[None, :]
    masks = np.stack([
        np.where(kq <= qq, 0.0, NEG).astype(np.float32),
        np.where(kq + 128 <= qq, 0.0, NEG).astype(np.float32),
    ])
    ones_col = np.ones((128, 1), np.float32)
    ones_row = np.ones((1, 128), bf16)
    ones_sq = np.ones((128, 128), bf16)

    # x: token-major waves [128, 2, 16, 512] per sequence
    xW_b = []
    for b in range(B):
        xT = x[b * S:(b + 1) * S].T                          # [HID, S]
        xW = xT.reshape(HID_T, 128, 2, 512).transpose(1, 2, 0, 3)
        xW_b.append(np.ascontiguousarray(xW).astype(bf16))

    in_maps = []
    for c in range(8):
        b, g = c // 2, c % 2
        wqb_t, wkb_t, wvb_t, wo_t = per_group[g]
        in_maps.append(dict(
            xW=xW_b[b], wqa_t=wqa_t, wkva_t=wkva_full, wkva_r=wkva_r,
            wqb_t=wqb_t, wkb_t=wkb_t, wvb_t=wvb_t, wo_t=wo_t,
            cosT=cosT, sinT=sinT, masks=masks,
            ones_col=ones_col, ones_row=ones_row, ones_sq=ones_sq))
    return in_maps


def assemble_output(results: list[dict]) -> np.ndarray:
    outs = []
    for b in range(B):
        outs.append(results[2 * b]["out"] + results[2 * b + 1]["out"])
    return np.concatenate(outs, axis=0)


# ---------------------------------------------------------------------------
# Harness entry point: full inputs in, full output out.
# ---------------------------------------------------------------------------

_NC_CACHE = []


def _get_nc():
    if not _NC_CACHE:
        _NC_CACHE.append(build_nc())
    return _NC_CACHE[0]


def kernel(_profile=False, **inputs) -> np.ndarray:
    """MLA attention on 8 NeuronCores: 4-way data-parallel over sequences x
    2-way tensor-parallel over heads. Takes full (unsharded) inputs, returns
    the full [4096, 2048] float32 output."""
    from concourse.bass_utils import run_bass_kernel_spmd

    seqstarts = np.asarray(inputs["seqstarts"])
    b = seqstarts.shape[0] - 1
    assert b == B and np.all(np.diff(seqstarts) == S), (
        "kernel compiled for 4 uniform sequences of 1024 tokens")

    nc = _get_nc()
    in_maps = prepare_inputs(inputs)
    kwargs = {}
    if _profile:
        _install_ntff_hook()
        kwargs = dict(trace=True, trace_cores=list(range(8)))
    res = run_bass_kernel_spmd(nc, in_maps, list(range(8)), **kwargs)
    out = assemble_output(res.results).astype(np.float32)
    if _profile:
        return out, res
    return out


def _install_ntff_hook():
    """The agent image lacks antenv.axon_hooks; reconstruct the NTFF profile
    hook via ctypes so run_bass_kernel_spmd(trace=True) works (profiling-only
    path, used by test.py)."""
    import types
    if 'antenv.axon_hooks' in sys.modules:
        return
    try:
        from trn_agent_boot.trn_boot import _ntff_profile_via_ctypes
        hook = _ntff_profile_via_ctypes('/opt/axon/libaxon_pjrt.so')
    except Exception:
        hook = None
    mod = types.ModuleType('antenv.axon_hooks')
    mod.get_axon_ntff_profile_hook = lambda: hook
    sys.modules['antenv.axon_hooks'] = mod


# revision 57
# speedup vs baseline: 1.1928x; 1.1928x over previous
"""MLA attention TRN2 kernel: 4-way data-parallel (sequences) x 2-way
tensor-parallel (heads). Each core: 1 sequence (1024 tokens), 8 heads.

v2: all matmul operands bf16 (fp32 PSUM accumulation), host-side
pre-tiled weights (contiguous per-partition DMA lines, ~6x fewer
descriptors), token-major x staging so q_a starts ~7us in, and
precomputed per-head rope-q / k_nope so the attention chunk loops
stream back-to-back on the PE (keeps the tensor engine at max p-state).

Layout convention: features on partitions, tokens on free axis; scores
computed transposed [k, q] so softmax sums use PE ones-matmuls and no
transposes are needed anywhere. Softmax denominator folded into avT via
per-(head, qblock-pair) broadcast multiply.
"""
import sys
sys.path.insert(0, '/opt/trn_rl_repo')

import math
import numpy as np

import concourse.bass as bass
import concourse.tile as tile
from concourse.tile_rust import add_dep_helper
from concourse import bacc, mybir

BF = mybir.dt.bfloat16
F32 = mybir.dt.float32
AF = mybir.ActivationFunctionType

H = 16
NH = 8            # heads per core
NOPE = 128
ROPE = 64
VD = 128
HID = 2048
QLR = 1536
KVLR = 512
B = 4
S = 1024
EPS = 1e-6
SCALE = 1.0 / math.sqrt(NOPE + ROPE)
NEG = -1.0e30

HID_T = HID // 128    # 16
QLR_T = QLR // 128    # 12
KVLR_T = KVLR // 128  # 4
QF = NH * (NOPE + ROPE)   # 1536 q features per core
QF_T = QF // 128          # 12 (chunks 0..7 nope, 8..11 rope)
NTC = S // 128            # 8 token chunks
QB = 256                  # query block
NQB = S // QB             # 4


def build_nc():
    nc = bacc.Bacc("TRN2", target_bir_lowering=False, debug=False, num_devices=8)

    xW = nc.dram_tensor("xW", [128, 2, HID_T, 512], BF, kind="ExternalInput")
    wqa_t = nc.dram_tensor("wqa_t", [QLR_T, 128, HID_T, 128], BF,
                           kind="ExternalInput")
    wkva_t = nc.dram_tensor("wkva_t", [4, 128, HID_T, 128], BF,
                            kind="ExternalInput")
    wkva_r = nc.dram_tensor("wkva_r", [128, HID_T, ROPE], BF,
                            kind="ExternalInput")
    wqb_t = nc.dram_tensor("wqb_t", [QF_T, 128, QLR_T, 128], BF,
                           kind="ExternalInput")
    wkb_t = nc.dram_tensor("wkb_t", [KVLR_T, 128, NH * NOPE], BF,
                           kind="ExternalInput")
    wvb_t = nc.dram_tensor("wvb_t", [KVLR_T, 128, NH * VD], BF,
                           kind="ExternalInput")
    wo_t = nc.dram_tensor("wo_t", [4, 128, NH, 512], BF, kind="ExternalInput")
    cosT = nc.dram_tensor("cosT", [ROPE // 2, S], BF, kind="ExternalInput")
    sinT = nc.dram_tensor("sinT", [ROPE // 2, S], BF, kind="ExternalInput")
    masks = nc.dram_tensor("masks", [2, 128, QB], F32, kind="ExternalInput")
    ones_col = nc.dram_tensor("ones_col", [128, 1], mybir.dt.float32r,
                              kind="ExternalInput")
    ones_row = nc.dram_tensor("ones_row", [1, 128], BF, kind="ExternalInput")
    ones_sq = nc.dram_tensor("ones_sq", [128, 128], BF, kind="ExternalInput")
    out = nc.dram_tensor("out", [S, HID], F32, kind="ExternalOutput")

    with tile.TileContext(nc) as tc:
        build_body(tc, xW=xW, wqa_t=wqa_t, wkva_t=wkva_t, wkva_r=wkva_r,
                   wqb_t=wqb_t, wkb_t=wkb_t, wvb_t=wvb_t, wo_t=wo_t,
                   cosT=cosT, sinT=sinT, masks=masks, ones_col=ones_col,
                   ones_row=ones_row, ones_sq=ones_sq, out=out)
    nc.compile()
    return nc


def build_body(tc, *, xW, wqa_t, wkva_t, wkva_r, wqb_t, wkb_t, wvb_t, wo_t,
               cosT, sinT, masks, ones_col, ones_row, ones_sq, out):
    from contextlib import ExitStack
    nc = tc.nc

    with (
        tc.tile_pool(name="const", bufs=1) as pconst,
        tc.tile_pool(name="ckv", bufs=1) as pckv,
    ):
        # const tiles created here; their DMAs are issued AFTER the
        # startup-critical x/wqa stream (none are needed before q_b).
        ones_c = pconst.tile([128, 1], mybir.dt.float32r, name="ones_c")
        ones_r = pconst.tile([1, 128], BF, name="ones_r")
        ones_s = pconst.tile([128, 128], BF, name="ones_s")
        mask_sb = [pconst.tile([128, QB], F32, name=f"mask{i}")
                   for i in range(2)]
        cos_sb = pconst.tile([ROPE // 2, S], BF, name="cos_sb")
        sin_sb = pconst.tile([ROPE // 2, S], BF, name="sin_sb")

        def emit_const_dmas():
            nc.sync.dma_start(ones_c[:], ones_col[:])
            nc.sync.dma_start(ones_r[:], ones_row[:])
            nc.sync.dma_start(ones_s[:], ones_sq[:])
            for i in range(2):
                nc.sync.dma_start(mask_sb[i][:], masks[i])
            nc.sync.dma_start(cos_sb[:], cosT[:])
            nc.sync.dma_start(sin_sb[:], sinT[:])
        # rope contract zero-padded to 128 partitions: 64-contract matmuls
        # measure ~280ns vs 216ns for 128-contract at 512 moving rows.
        kpe = pconst.tile([128, S], BF, name="kpe")
        nc.vector.memset(kpe[ROPE:128, :], 0.0)
        eps_t = pconst.tile([1, 1], F32, name="eps_t")
        nc.vector.memset(eps_t[:], EPS)

        ckv = [pckv.tile([128, S], BF, name=f"ckv{i}") for i in range(KVLR_T)]

        # ============ Phases: q_a -> kv_a (norms overlap) -> q_b ============
        es_u = ExitStack()
        pu = es_u.enter_context(tc.tile_pool(name="u", bufs=1))
        es_qbc = ExitStack()
        pqbc = es_qbc.enter_context(tc.tile_pool(name="qbc", bufs=1))
        qbc = [pqbc.tile([128, 512], BF, name=f"qbcn{i}") for i in range(2)]
        # cross-phase rmsnorm state: f32 square-sum accumulators (filled on
        # DVE during q_a/kv_a; reduced + broadcast inside the q_b stream) and
        # the raw k_pe rows.
        es_nrm = ExitStack()
        pnrm2 = es_nrm.enter_context(tc.tile_pool(name="nrm2", bufs=1))
        z2qa = [pnrm2.tile([128, 512], mybir.dt.float32r, name=f"z2qa{h}")
                for h in range(2)]
        z2ka = [pnrm2.tile([128, 512], mybir.dt.float32r, name=f"z2ka{h}")
                for h in range(2)]
        kv4 = pnrm2.tile([ROPE, S], BF, name="kv4")
        es_x = ExitStack()
        px = es_x.enter_context(tc.tile_pool(name="xsb", bufs=1))

        # token-major x staging: wave w = tokens [w*512, (w+1)*512) for all
        # of HID, so q_a half 0 can start as soon as wave 0 lands.
        xsb = px.tile([128, 2, HID_T, 512], BF, name="xsb")

        es_wqa = ExitStack()
        pwqa = es_wqa.enter_context(tc.tile_pool(name="wqa", bufs=1))
        wqa_s = [pwqa.tile([128, HID_T, 128], BF, name=f"wqa_s{m}")
                 for m in range(QLR_T)]
        # startup-critical DMA order: interleave strip0/wave0 in ko-quarters
        # (the first q_a group consumes ko 0..15 serially), then strip 1,
        # strips 2-11, consts, wave 1.
        for sub in range(4):
            ksl = slice(4 * sub, 4 * sub + 4)
            nc.sync.dma_start(wqa_s[0][:, ksl, :], wqa_t[0][:, ksl, :])
            nc.sync.dma_start(xsb[:, 0, ksl, :], xW[:, 0, ksl, :])
        nc.sync.dma_start(wqa_s[1][:], wqa_t[1])
        for m in range(2, QLR_T):
            nc.sync.dma_start(wqa_s[m][:], wqa_t[m])
        emit_const_dmas()
        nc.sync.dma_start(xsb[:, 1], xW[:, 1])

        u = [pu.tile([128, S], BF, name=f"u{i}") for i in range(QLR_T)]

        with (
            tc.tile_pool(name="wstripA", bufs=3) as pwA,
            tc.tile_pool(name="sqA", bufs=4) as psq,
            tc.tile_pool(name="ppmain", bufs=4, space="PSUM") as ppm,
            tc.tile_pool(name="ppwarm", bufs=1, space="PSUM") as ppw,
        ):
            # ---- PE p-state warm-up: run dummy matmuls on a memset tile
            # while the startup DMAs land, so the tensor engine is at max
            # clock (needs ~3us continuous busy) when q_a starts. ----
            dz = psq.tile([128, 512], BF, name="dz", tag="warmz", bufs=1)
            nc.vector.memset(dz[:], 0.0)
            ps_w = ppw.tile([128, 512], F32, name="ps_warm", tag="warm")
            for i in range(10):
                nc.tensor.matmul(ps_w[:], dz[:, 0:128], dz[:],
                                 skip_group_check=True)

            # ---- q_a: u = wqa.T @ x -> [1536, S], half-major over tokens ----
            # square-sums accumulate on DVE (f32 SBUF acc); the partition
            # reduce + rsqrt + broadcast run later, inside the q_b stream.
            with nc.named_scope("q_a"):
                for half in range(2):
                    sl = slice(half * 512, (half + 1) * 512)
                    for m in range(QLR_T):
                        ps = ppm.tile([128, 512], F32, name=f"psu{m}_{half}",
                                      tag="main")
                        for ko in range(HID_T):
                            nc.tensor.matmul(
                                ps[:], wqa_s[m][:, ko, :], xsb[:, half, ko, :],
                                start=(ko == 0), stop=(ko == HID_T - 1))
                        cp = nc.scalar.copy(u[m][:, sl], ps[:])
                        if m == 0 and half == 0:
                            gate_qa = cp
                        sq = psq.tile([128, 512], BF, name=f"squ{m}_{half}",
                                      tag="sq")
                        nc.scalar.activation(sq[:], ps[:], AF.Square)
                        if m == 0:
                            nc.vector.tensor_copy(z2qa[half][:], sq[:])
                        else:
                            nc.vector.tensor_add(z2qa[half][:], z2qa[half][:],
                                                 sq[:])

            # ---- kv_a: kvT = wkva.T @ x -> [576, S] ----
            with nc.named_scope("kv_a"):
                for m in range(5):
                    mw = 128 if m < 4 else ROPE
                    ws = pwA.tile([128, HID_T, mw], BF, name=f"wkva_s{m}",
                                  tag="wstrip")
                    nc.sync.dma_start(ws[:], wkva_t[m] if m < 4 else wkva_r[:])
                    for half in range(2):
                        sl = slice(half * 512, (half + 1) * 512)
                        ps = ppm.tile([128, 512], F32, name=f"pskv{m}_{half}",
                                      tag="main")
                        for ko in range(HID_T):
                            nc.tensor.matmul(
                                ps[:mw, :], ws[:, ko, :], xsb[:, half, ko, :],
                                start=(ko == 0), stop=(ko == HID_T - 1))
                        if m < 4:
                            nc.scalar.copy(ckv[m][:, sl], ps[:])
                            sq = psq.tile([128, 512], BF, name=f"sqk{m}_{half}",
                                          tag="sq")
                            sq_act = nc.scalar.activation(sq[:], ps[:],
                                                          AF.Square)
                            if m == 3:
                                last_square = sq_act
                            if m == 0:
                                nc.vector.tensor_copy(z2ka[half][:], sq[:])
                            else:
                                nc.vector.tensor_add(z2ka[half][:],
                                                     z2ka[half][:], sq[:])
                        else:
                            nc.scalar.copy(kv4[:, sl], ps[:mw, :])
                # pre-warm the rsqrt activation table after the last Square
                # so norm_stage's rsqrt does not pay the ~1.3us table load
                rsw = psq.tile([1, 1], BF, name="rsqwarm", tag="rsw", bufs=1)
                warm = nc.scalar.activation(rsw[:], eps_t[:],
                                            AF.Abs_reciprocal_sqrt)
                add_dep_helper(warm.ins, last_square.ins, sync=True,
                               reason="rsqrt table warm-up after last Square")

        # ---- q_b: qT = wqb.T @ u -> [1536, S] (x, wqa freed). The rmsnorm
        # reduce/rsqrt/broadcast work is staggered into the first three q_b
        # groups so the PE never waits on the scalar-engine norm chain. ----
        es_wqa.close()
        es_x.close()
        es_qT = ExitStack()
        pqT = es_qT.enter_context(tc.tile_pool(name="qT", bufs=1, side="right"))
        qT = [pqT.tile([128, S], BF, name=f"qTt{i}") for i in range(QF_T)]
        with (
            tc.tile_pool(name="wstripB", bufs=3) as pwB,
            tc.tile_pool(name="normB", bufs=2) as pnorm,
            tc.tile_pool(name="ppmainB", bufs=4, space="PSUM") as ppmB,
            tc.tile_pool(name="ppz2", bufs=2, space="PSUM") as ppz2,
            tc.tile_pool(name="ppbc", bufs=2, space="PSUM") as ppbc,
        ):
            z2q = [ppz2.tile([1, 512], F32, name=f"z2q{h}", tag="z2")
                   for h in range(2)]
            z2k = [ppz2.tile([1, 512], F32, name=f"z2k{h}", tag="z2")
                   for h in range(2)]
            rrq = [pnorm.tile([1, 512], BF, name=f"qrr{h}", tag="rr")
                   for h in range(2)]
            rrk = [pnorm.tile([1, 512], BF, name=f"krr{h}", tag="rrk")
                   for h in range(2)]

            def emit_rsqrt(rr, z2, n, nm):
                nc.scalar.activation(rr[:], z2[:], AF.Abs_reciprocal_sqrt,
                                     scale=1.0 / n, bias=eps_t[:])

            def norm_stage(m):
                if m == 0:
                    # partition-reduce q square-sums; rsqrt chain
                    for half in range(2):
                        nc.tensor.matmul(z2q[half][:], ones_c[:],
                                         z2qa[half][:], skip_group_check=True)
                        emit_rsqrt(rrq[half], z2q[half], QLR, f'q{half}')
                elif m == 5:
                    for half in range(2):
                        pb = ppbc.tile([128, 512], F32, name=f"qpb{half}",
                                       tag="bc")
                        nc.tensor.matmul(pb[:], ones_r[:], rrq[half][:],
                                         skip_group_check=True)
                        nc.scalar.copy(qbc[half][:], pb[:])
                        nc.tensor.matmul(z2k[half][:], ones_c[:],
                                         z2ka[half][:], skip_group_check=True)
                        emit_rsqrt(rrk[half], z2k[half], KVLR, f'k{half}')
                elif m == 8:
                    with nc.named_scope("ckv_norm"):
                        for half in range(2):
                            sl = slice(half * 512, (half + 1) * 512)
                            pb = ppbc.tile([128, 512], F32, name=f"kpb{half}",
                                           tag="bc")
                            nc.tensor.matmul(pb[:], ones_r[:], rrk[half][:],
                                             skip_group_check=True)
                            bc = pnorm.tile([128, 512], BF, name=f"kbc{half}",
                                            tag="kbc")
                            nc.scalar.copy(bc[:], pb[:])
                            for mm in range(KVLR_T):
                                nc.vector.tensor_mul(ckv[mm][:, sl],
                                                     ckv[mm][:, sl], bc[:])
                        # RoPE for k_pe; DVE needs equal base partitions
                        e = kv4[0:32, :]
                        ko0 = pnorm.tile([32, S], BF, name="ko0", tag="r32a",
                                         bufs=1)
                        nc.gpsimd.dma_start(ko0[:], kv4[32:64, :])
                        kt1 = pnorm.tile([32, S], BF, name="kt1", tag="r32b",
                                         bufs=1)
                        nc.vector.tensor_mul(kt1[:], e, cos_sb[:])
                        kt2 = pnorm.tile([32, S], BF, name="kt2", tag="r32c",
                                         bufs=1)
                        nc.vector.tensor_mul(kt2[:], ko0[:], sin_sb[:])
                        nc.vector.tensor_sub(kpe[0:32, :], kt1[:], kt2[:])
                        kt3 = pnorm.tile([32, S], BF, name="kt3", tag="r32b",
                                         bufs=1)
                        nc.vector.tensor_mul(kt3[:], e, sin_sb[:])
                        kt4 = pnorm.tile([32, S], BF, name="kt4", tag="r32c",
                                         bufs=1)
                        nc.vector.tensor_mul(kt4[:], ko0[:], cos_sb[:])
                        ktO = pnorm.tile([32, S], BF, name="ktO", tag="r32a",
                                         bufs=1)
                        nc.vector.tensor_add(ktO[:], kt3[:], kt4[:])
                        nc.gpsimd.dma_start(kpe[32:64, :], ktO[:])

            with nc.named_scope("q_b"):
                # qbc is produced at norm_stage(1); the norm muls for the
                # first two strips must be EMITTED after that write (an
                # earlier-emitted read would be a WAR on stale qbc).
                mul_pend = []
                for m in range(QF_T):
                    ws = pwB.tile([128, QLR_T, 128], BF, name=f"wqb_s{m}",
                                  tag="wstripB")
                    dma = nc.sync.dma_start(ws[:], wqb_t[m])
                    add_dep_helper(dma.ins, gate_qa.ins, sync=True,
                                   reason="stage wqb DMA after q_a starts")
                    norm_stage(m)
                    if m == 6:
                        for fn in mul_pend:
                            fn()
                        mul_pend = []
                    if m == 7:
                        # pre-warm the EXP activation table off the critical
                        # path (the load costs ~1.3us on the scalar engine)
                        warm = pnorm.tile([1, 512], BF, name="expwarm",
                                          tag="rt", bufs=1)
                        nc.scalar.activation(warm[:], z2q[0][:], AF.Exp,
                                             scale=SCALE)
                    for half in range(2):
                        sl = slice(half * 512, (half + 1) * 512)
                        ps = ppmB.tile([128, 512], F32, name=f"psq{m}_{half}",
                                       tag="mainB")
                        for ko in range(QLR_T):
                            nc.tensor.matmul(
                                ps[:], ws[:, ko, :], u[ko][:, sl],
                                start=(ko == 0), stop=(ko == QLR_T - 1))
                        cp = nc.scalar.copy(qT[m][:, sl], ps[:])
                        if m < 6:
                            mul_pend.append(
                                lambda m=m, sl=sl, half=half:
                                nc.vector.tensor_mul(qT[m][:, sl],
                                                     qT[m][:, sl],
                                                     qbc[half][:]))
                        else:
                            nc.vector.tensor_mul(qT[m][:, sl], qT[m][:, sl],
                                                 qbc[half][:])
                        if m == 0 and half == 0:
                            gate_qb = cp
        es_nrm.close()
        es_qbc.close()
        es_u.close()

        # pools in reverse-close order: avT (o_proj) under qpeA/knA
        # (attention) under qrope (released right after q_rope consumers).
        es_avT = ExitStack()
        pavT = es_avT.enter_context(tc.tile_pool(name="avT", bufs=1))
        avT = [pavT.tile([128, S], BF, name=f"avT{i}") for i in range(NH)]
        es_qpe = ExitStack()
        pqpeA = es_qpe.enter_context(tc.tile_pool(name="qpeA", bufs=1))
        qpe_all = pqpeA.tile([128, NH, S], BF, name="qpe_all")
        nc.vector.memset(qpe_all[ROPE:128], 0.0)
        es_knA = ExitStack()
        pknA = es_knA.enter_context(tc.tile_pool(name="knA", bufs=1))
        kn_all = pknA.tile([128, NH, S], BF, name="kn_all")
        es_qr = ExitStack()
        pqr = es_qr.enter_context(tc.tile_pool(name="qrope", bufs=1))

        # ---- RoPE for all heads' q_pe upfront (overlaps v_b/kn below) ----
        with nc.named_scope("q_rope"):
            # 4 heads stacked on partitions (32 rows each), 2 head-groups on
            # the free axis: qe4[32*i:32*i+32, jj, :] = evens of head jj*4+i.
            qe4 = pqr.tile([128, 2, S], BF, name="qe4")
            qo4 = pqr.tile([128, 2, S], BF, name="qo4")
            c4 = pqr.tile([128, 2, S], BF, name="c4")
            s4 = pqr.tile([128, 2, S], BF, name="s4")
            for i in range(4):
                for jj in range(2):
                    nc.gpsimd.dma_start(c4[32 * i:32 * i + 32, jj, :],
                                        cos_sb[:])
                    nc.gpsimd.dma_start(s4[32 * i:32 * i + 32, jj, :],
                                        sin_sb[:])
            for h in range(NH):
                j, b2 = h // 2, h % 2
                base = b2 * 64
                i, jj = h % 4, h // 4
                nc.gpsimd.dma_start(qe4[32 * i:32 * i + 32, jj, :],
                                    qT[NH + j][base:base + 32, :])
                nc.gpsimd.dma_start(qo4[32 * i:32 * i + 32, jj, :],
                                    qT[NH + j][base + 32:base + 64, :])
            ta = pqr.tile([128, 2, S], BF, name="qta", tag="ta", bufs=1)
            nc.vector.tensor_mul(ta[:], qe4[:], c4[:])
            tb = pqr.tile([128, 2, S], BF, name="qtb", tag="tb", bufs=1)
            nc.vector.tensor_mul(tb[:], qo4[:], s4[:])
            qpeE = pqr.tile([128, 2, S], BF, name="qpeE")
            nc.vector.tensor_sub(qpeE[:], ta[:], tb[:])
            ta2 = pqr.tile([128, 2, S], BF, name="qta2", tag="ta", bufs=1)
            nc.vector.tensor_mul(ta2[:], qe4[:], s4[:])
            tb2 = pqr.tile([128, 2, S], BF, name="qtb2", tag="tb", bufs=1)
            nc.vector.tensor_mul(tb2[:], qo4[:], c4[:])
            qpeO = pqr.tile([128, 2, S], BF, name="qpeO")
            nc.vector.tensor_add(qpeO[:], ta2[:], tb2[:])
            for h in range(NH):
                i, jj = h % 4, h // 4
                nc.gpsimd.dma_start(qpe_all[0:32, h, :],
                                    qpeE[32 * i:32 * i + 32, jj, :])
                nc.gpsimd.dma_start(qpe_all[32:64, h, :],
                                    qpeO[32 * i:32 * i + 32, jj, :])

        # ======================= attention =======================
        with tc.tile_pool(name="wkb", bufs=1) as pwkb:
            wkb = [pwkb.tile([128, NH * NOPE], BF, name=f"wkb{i}")
                   for i in range(KVLR_T)]
            for ko in range(KVLR_T):
                dma = nc.sync.dma_start(wkb[ko][:], wkb_t[ko])
                add_dep_helper(dma.ins, gate_qb.ins, sync=True,
                               reason="stage wkb DMA after q_b starts")

            with tc.tile_pool(name="v", bufs=1) as pv:
                v_sb = [[pv.tile([128, 512], BF, name=f"v{g}_{t}")
                         for t in range(NTC)] for g in range(2)]
                with (
                    tc.tile_pool(name="wvb", bufs=1) as pwvb,
                    tc.tile_pool(name="ppv", bufs=4, space="PSUM") as ppv,
                    tc.tile_pool(name="ppknb", bufs=2, space="PSUM") as ppknb,
                ):
                    wvb = [pwvb.tile([128, NH * VD], BF, name=f"wvb{i}")
                           for i in range(KVLR_T)]
                    for ko in range(KVLR_T):
                        dma = nc.sync.dma_start(wvb[ko][:], wvb_t[ko])
                        add_dep_helper(dma.ins, gate_qb.ins, sync=True,
                                       reason="stage wvb DMA after q_b starts")
                    with nc.named_scope("v_b"):
                        for g in range(2):
                            for t in range(NTC):
                                ps = ppv.tile([128, 512], F32,
                                              name=f"psv{g}_{t}", tag="v")
                                for kk in range(KVLR_T):
                                    nc.tensor.matmul(
                                        ps[:],
                                        ckv[kk][:, t * 128:(t + 1) * 128],
                                        wvb[kk][:, g * 512:(g + 1) * 512],
                                        start=(kk == 0),
                                        stop=(kk == KVLR_T - 1))
                                nc.scalar.copy(v_sb[g][t][:], ps[:])
                    with nc.named_scope("k_b"):
                        for h in range(NH):
                            for half in range(2):
                                sl = slice(half * 512, (half + 1) * 512)
                                ps = ppknb.tile([128, 512], F32,
                                                name=f"pskn{h}_{half}",
                                                tag="kn")
                                for kk in range(KVLR_T):
                                    nc.tensor.matmul(
                                        ps[:],
                                        wkb[kk][:, h * 128:(h + 1) * 128],
                                        ckv[kk][:, sl],
                                        start=(kk == 0),
                                        stop=(kk == KVLR_T - 1))
                                nc.vector.tensor_copy(kn_all[:, h, sl], ps[:])

                with (
                    tc.tile_pool(name="pbuf", bufs=4) as ppbuf,
                    tc.tile_pool(name="zbuf", bufs=2) as pzbuf,
                    tc.tile_pool(name="ppsc", bufs=4, space="PSUM") as ppsc,
                    tc.tile_pool(name="ppav", bufs=2, space="PSUM") as ppav,
                    tc.tile_pool(name="ppz", bufs=2, space="PSUM") as ppz,
                ):
                    # the last chunk's av/z matmuls and pair-1 divide of each
                    # head carry into the next head's emission so the PE does
                    # not wait on the final exp at head boundaries
                    carry = []
                    for h in range(NH):
                        with nc.named_scope(f"attn_h{h}"):
                            amul, carry = attention_head(
                                tc, h, qT=qT, kn_all=kn_all, kpe=kpe,
                                qpe_all=qpe_all, v_sb=v_sb, avT=avT,
                                mask_sb=mask_sb, ones_s=ones_s,
                                ppbuf=ppbuf, pzbuf=pzbuf, ppsc=ppsc,
                                ppav=ppav, ppz=ppz, carry=carry)
                            if h == 0:
                                gate_attn = amul
                    for fn in carry:
                        fn()

                    # ====== o_proj (inside the attention pool scope so its
                    # PSUM tiles rotate through the score banks — freed by
                    # the last head's exps, not its divide chain — and no
                    # pool-transition barrier sits on the handoff) ======
                    with (
                        tc.tile_pool(name="wo", bufs=4) as pwo,
                        tc.tile_pool(name="osb", bufs=3) as posb,
                    ):
                        with nc.named_scope("o_proj"):
                            for hc in range(4):
                                ws = pwo.tile([128, NH, 512], BF,
                                              name=f"wo_s{hc}", tag="wo")
                                dma = nc.sync.dma_start(ws[:], wo_t[hc])
                                add_dep_helper(
                                    dma.ins, gate_attn.ins, sync=True,
                                    reason="stage wo DMA after attention "
                                           "starts")
                                for t in range(NTC):
                                    ps = ppsc.tile([128, 512], F32,
                                                   name=f"pso{hc}_{t}",
                                                   tag="sc")
                                    for kk in range(NH):
                                        nc.tensor.matmul(
                                            ps[:],
                                            avT[kk][:, t * 128:(t + 1) * 128],
                                            ws[:, kk, :],
                                            start=(kk == 0),
                                            stop=(kk == NH - 1))
                                    ot = posb.tile([128, 512], F32,
                                                   name=f"ot{hc}_{t}",
                                                   tag="ot")
                                    nc.scalar.copy(ot[:], ps[:])
                                    nc.gpsimd.dma_start(
                                        out[t * 128:(t + 1) * 128,
                                            hc * 512:(hc + 1) * 512], ot[:])
        es_qr.close()
        es_knA.close()
        es_qpe.close()
        es_qT.close()
        es_avT.close()


def attention_head(tc, h, *, qT, kn_all, kpe, qpe_all, v_sb, avT, mask_sb,
                   ones_s, ppbuf, pzbuf, ppsc, ppav, ppz, carry):
    nc = tc.nc

    # Two query-block pairs; pair p covers qblocks (2p, 2p+1) at columns
    # p*512..(p+1)*512. Chunk loop shared so the kn/kpe stationaries are
    # reused back-to-back across pairs.
    st = []
    for p in range(2):
        q0 = 2 * p
        nw = 2 * q0 + 2
        st.append(dict(
            q0=q0, nw=nw, nk=nw + 2,
            psl=slice(p * 512, (p + 1) * 512),
            nsl=slice(p * 512 + 256, p * 512 + 512),
            ps_av=ppav.tile([128, 512], F32, name=f"psav{h}_{p}", tag="av"),
            ps_z=ppz.tile([128, 512], F32, name=f"psz{h}_{p}", tag="z"),
        ))
    pend = list(carry)

    def finish_chunk(p, kc, ps_s):
        s = st[p]
        wide = kc < s["nw"]
        cn = 512 if wide else 256
        d = kc - 2 * s["q0"] if wide else kc - 2 * (s["q0"] + 1)
        if d >= 0:
            nc.vector.tensor_add(ps_s[:, 0:256], ps_s[:, 0:256],
                                 mask_sb[d][:])
        p_sb = ppbuf.tile([128, 512], BF, name=f"p{h}_{p}_{kc}", tag="p")
        nc.scalar.activation(p_sb[:, :cn], ps_s[:, :cn], AF.Exp, scale=SCALE)

        def avz():
            vt = v_sb[h // 4][kc][:, (h % 4) * 128:(h % 4 + 1) * 128]
            osl = slice(0, 512) if wide else slice(256, 512)
            nc.tensor.matmul(s["ps_av"][:, osl], vt, p_sb[:, :cn],
                             start=(kc == 0), stop=(kc == s["nk"] - 1),
                             skip_group_check=True)
            nc.tensor.matmul(s["ps_z"][:, osl], ones_s[:], p_sb[:, :cn],
                             start=(kc == 0), stop=(kc == s["nk"] - 1),
                             skip_group_check=True)
        return avz

    def divide(p):
        s = st[p]
        zr = pzbuf.tile([128, 512], F32, name=f"zr{h}_{p}", tag="zr")
        nc.vector.reciprocal_approx_fast(zr[:], s["ps_z"][:])
        return nc.vector.tensor_mul(avT[h][:, s["psl"]], s["ps_av"][:],
                                    zr[:])

    for kc in range(st[1]["nk"]):
        ksl = slice(kc * 128, (kc + 1) * 128)
        active = [p for p in range(2) if kc < st[p]["nk"]]
        tiles = {}
        # nope score matmuls first (shared kn stationary), then rope
        # (shared kpe stationary).
        for p in active:
            s = st[p]
            wide = kc < s["nw"]
            cn = 512 if wide else 256
            csl = s["psl"] if wide else s["nsl"]
            ps_s = ppsc.tile([128, 512], F32, name=f"pss{h}_{p}_{kc}",
                             tag="sc")
            tiles[p] = ps_s
            nc.tensor.matmul(ps_s[:, :cn], kn_all[:, h, ksl],
                             qT[h][:, csl], start=True, stop=False,
                             skip_group_check=True)
        for p in active:
            s = st[p]
            wide = kc < s["nw"]
            cn = 512 if wide else 256
            csl = s["psl"] if wide else s["nsl"]
            nc.tensor.matmul(tiles[p][:, :cn], kpe[:, ksl],
                             qpe_all[:, h, csl], start=False, stop=True,
                             skip_group_check=True)
        cur = [finish_chunk(p, kc, tiles[p]) for p in active]
        for fn in pend:
            fn()
        pend = cur
        if kc == st[0]["nk"]:
            # pair 0's accumulation completed in the flush above; divide it
            # now so its PSUM banks free early (shortens the o_proj handoff)
            mul0 = divide(0)
    leftover = list(pend) + [lambda: divide(1)]
    return mul0, leftover


# ---------------------------------------------------------------------------
# Host-side prep
# ---------------------------------------------------------------------------

def _strip_tile(wT, sw):
    """[K, W] -> [W//sw, 128, K//128, sw]: per-strip contiguous SBUF images."""
    K, W = wT.shape
    ko_n = K // 128
    n = W // sw
    return np.ascontiguousarray(
        wT.reshape(ko_n, 128, n, sw).transpose(2, 1, 0, 3))


def prepare_inputs(inputs: dict) -> list[dict]:
    """Full problem inputs -> list of 8 per-core input maps."""
    import ml_dtypes
    bf16 = ml_dtypes.bfloat16

    x = np.asarray(inputs["x"], np.float32)
    wq_a = np.asarray(inputs["wq_a"], np.float32)
    w_qa_ln = np.asarray(inputs["w_qa_ln"], np.float32)
    wq_b = np.asarray(inputs["wq_b"], np.float32)
    wkv_a = np.asarray(inputs["wkv_a"], np.float32)
    w_kva_ln = np.asarray(inputs["w_kva_ln"], np.float32)
    wk_b = np.asarray(inputs["wk_b"], np.float32)
    wv_b = np.asarray(inputs["wv_b"], np.float32)
    wo = np.asarray(inputs["wo"], np.float32)
    rotary_sin = np.asarray(inputs["rotary_sin"], np.float32)
    rotary_cos = np.asarray(inputs["rotary_cos"], np.float32)

    wqaT = np.ascontiguousarray(wq_a.T)                      # [HID, QLR]
    kv_perm = (list(range(KVLR))
               + [KVLR + 2 * i for i in range(ROPE // 2)]
               + [KVLR + 2 * i + 1 for i in range(ROPE // 2)])
    wkvaT = np.ascontiguousarray(wkv_a[kv_perm, :].T)        # [HID, 576]

    wq_b_eff = wq_b * w_qa_ln[None, :]
    wk_b_eff = wk_b * w_kva_ln[None, :]
    wv_b_eff = wv_b * w_kva_ln[None, :]

    wqa_t = _strip_tile(wqaT, 128).astype(bf16)              # [12,128,16,128]
    wkva_full = _strip_tile(wkvaT[:, :KVLR], 128).astype(bf16)  # [4,128,16,128]
    wkva_r = _strip_tile(wkvaT[:, KVLR:], ROPE)[0].astype(bf16)  # [128,16,64]

    per_group = []
    for g in range(2):
        heads = range(g * NH, (g + 1) * NH)
        qperm = [h * (NOPE + ROPE) + d for h in heads for d in range(NOPE)]
        for h in heads:
            qperm += [h * (NOPE + ROPE) + NOPE + 2 * i for i in range(ROPE // 2)]
            qperm += [h * (NOPE + ROPE) + NOPE + 2 * i + 1
                      for i in range(ROPE // 2)]
        wqbT = np.ascontiguousarray(wq_b_eff[qperm, :].T)    # [QLR, 1536]
        wqb_t = _strip_tile(wqbT, 128).astype(bf16)          # [12,128,12,128]
        cols = [h * NOPE + d for h in heads for d in range(NOPE)]
        wkbT = wk_b_eff[cols, :].T                           # [KVLR, 1024]
        wkb_t = np.ascontiguousarray(
            wkbT.reshape(KVLR_T, 128, NH * NOPE)).astype(bf16)
        wvbT = wv_b_eff[cols, :].T
        wvb_t = np.ascontiguousarray(
            wvbT.reshape(KVLR_T, 128, NH * VD)).astype(bf16)
        woT = np.ascontiguousarray(wo[:, cols].T)            # [1024, HID]
        wo_t = _strip_tile(woT, 512).astype(bf16)            # [4,128,8,512]
        wo_t = np.ascontiguousarray(wo_t)
        per_group.append((wqb_t, wkb_t, wvb_t, wo_t))

    cosT = np.ascontiguousarray(rotary_cos.T).astype(bf16)   # [32, S]
    sinT = np.ascontiguousarray(rotary_sin.T).astype(bf16)
    kq = np.arange(128)[:, None]
    qq = np.arange(QB)[None, :]
    masks = np.stack([
        np.where(kq <= qq, 0.0, NEG).astype(np.float32),
        np.where(kq + 128 <= qq, 0.0, NEG).astype(np.float32),
    ])
    ones_col = np.ones((128, 1), np.float32)
    ones_row = np.ones((1, 128), bf16)
    ones_sq = np.ones((128, 128), bf16)

    # x: token-major waves [128, 2, 16, 512] per sequence
    xW_b = []
    for b in range(B):
        xT = x[b * S:(b + 1) * S].T                          # [HID, S]
        xW = xT.reshape(HID_T, 128, 2, 512).transpose(1, 2, 0, 3)
        xW_b.append(np.ascontiguousarray(xW).astype(bf16))

    in_maps = []
    for c in range(8):
        b, g = c // 2, c % 2
        wqb_t, wkb_t, wvb_t, wo_t = per_group[g]
        in_maps.append(dict(
            xW=xW_b[b], wqa_t=wqa_t, wkva_t=wkva_full, wkva_r=wkva_r,
            wqb_t=wqb_t, wkb_t=wkb_t, wvb_t=wvb_t, wo_t=wo_t,
            cosT=cosT, sinT=sinT, masks=masks,
            ones_col=ones_col, ones_row=ones_row, ones_sq=ones_sq))
    return in_maps


def assemble_output(results: list[dict]) -> np.ndarray:
    outs = []
    for b in range(B):
        outs.append(results[2 * b]["out"] + results[2 * b + 1]["out"])
    return np.concatenate(outs, axis=0)


# ---------------------------------------------------------------------------
# Harness entry point: full inputs in, full output out.
# ---------------------------------------------------------------------------

_NC_CACHE = []


def _get_nc():
    if not _NC_CACHE:
        _NC_CACHE.append(build_nc())
    return _NC_CACHE[0]


def kernel(_profile=False, **inputs) -> np.ndarray:
    """MLA attention on 8 NeuronCores: 4-way data-parallel over sequences x
    2-way tensor-parallel over heads. Takes full (unsharded) inputs, returns
    the full [4096, 2048] float32 output."""
    from concourse.bass_utils import run_bass_kernel_spmd

    seqstarts = np.asarray(inputs["seqstarts"])
    b = seqstarts.shape[0] - 1
    assert b == B and np.all(np.diff(seqstarts) == S), (
        "kernel compiled for 4 uniform sequences of 1024 tokens")

    nc = _get_nc()
    in_maps = prepare_inputs(inputs)
    kwargs = {}
    if _profile:
        _install_ntff_hook()
        kwargs = dict(trace=True, trace_cores=list(range(8)))
    res = run_bass_kernel_spmd(nc, in_maps, list(range(8)), **kwargs)
    out = assemble_output(res.results).astype(np.float32)
    if _profile:
        return out, res
    return out


def _install_ntff_hook():
    """The agent image lacks antenv.axon_hooks; reconstruct the NTFF profile
    hook via ctypes so run_bass_kernel_spmd(trace=True) works (profiling-only
    path, used by test.py)."""
    import types
    if 'antenv.axon_hooks' in sys.modules:
        return
    try:
        from trn_agent_boot.trn_boot import _ntff_profile_via_ctypes
        hook = _ntff_profile_via_ctypes('/opt/axon/libaxon_pjrt.so')
    except Exception:
        hook = None
    mod = types.ModuleType('antenv.axon_hooks')
    mod.get_axon_ntff_profile_hook = lambda: hook
    sys.modules['antenv.axon_hooks'] = mod
